# revision 8
# baseline (speedup 1.0000x reference)
"""ChebyKAN layer kernel for 8x Trainium2 NeuronCores.

Computes y[b,o] = sum_{i,d} T_d(tanh(x[b,i])) * C[i,o,d], d = 0..8,
with T_d the Chebyshev polynomials, via:
  - batch sharded 8 ways (1024 rows/core)
  - device computes T_1..T_8 with Chebyshev product identities
    (fp32 DVE/ACT)
  - d=0 term (T_0 == 1) folded into a host-precomputed bias[o]
  - contraction as matmuls accumulating fp32 in PSUM, K = (i,d) of
    size 8192:
      * i-chunks 0..5 (K=6144): bf16 basis x bf16 weights
      * i-chunks 6..7 (K=2048): fp8-e4m3 basis x fp8-e4m3 weights in
        DoubleRow perf mode (2 K-chunks per PE pass, ~1.44x bf16
        throughput). Quantization noise of the fp8 quarter keeps the
        full-output max rel err ~1.5e-2 (measured on the fixed
        setup_inputs seed), under the 2e-2 gate.
  - all weights host-scaled by 2^12 (exact in bf16/fp8) so the fp8
    coeffs (~1e-4) sit in e4m3's normal range; the drain rescales by
    2^-12 before adding the bias.
  - x is transposed on host so the basis is produced directly in
    [K, batch] (lhsT) layout; no on-device transpose needed.

Self-contained: hardcodes all shapes for inputs
  x: [8192, 1024] f32, cheby_coeffs: [1024, 1024, 9] f32.
"""

import numpy as np
import ml_dtypes

import concourse.bass as bass
import concourse.mybir as mybir
import concourse.tile as tile
from concourse import bacc
from concourse.bass_utils import run_bass_kernel_spmd

P = 128
B_TOTAL = 8192
I_DIM = 1024
O_DIM = 1024
DEG = 8              # degrees 1..8 on device (d=0 folded into bias)
N_CORES = 8
B_LOCAL = B_TOTAL // N_CORES     # 1024
IC = I_DIM // P                  # 8 input chunks
IC_BF = 5                        # i-chunks contracted in bf16
IC_F8 = IC - IC_BF               # i-chunks contracted in dual-fp8
NK_BF = IC_BF * DEG              # 40 bf16 K-chunks of 128
NPAIR = IC_F8 * (DEG // 2)       # 12 fp8 DoubleRow pairs (K=256 each)
OH = 2                           # output halves (PSUM capacity: 8 banks)
ON = O_DIM // OH                 # 512
W_SCALE = 4096.0                 # pow2: exact in bf16; lifts fp8 coeffs
W_SINV = 1.0 / W_SCALE           # out of e4m3's subnormal range

_nc = None
last_results = None  # BassKernelResults of the most recent run (for profiling)


def _ensure_ntff_hook():
    """bass_utils' trace path imports antenv.axon_hooks unconditionally, but
    this agent image's antenv package lacks that module. Synthesize it (with
    the real libaxon NTFF hook when available) so a BASS_TRACE=1 run traces
    instead of crashing."""
    import sys
    import types

    try:
        import antenv.axon_hooks  # noqa: F401
        return
    except ImportError:
        pass
    try:
        import antenv
    except ImportError:
        return
    hook = None
    try:
        from trn_agent_boot.trn_boot import _ntff_profile_via_ctypes
        hook = _ntff_profile_via_ctypes("/opt/axon/libaxon_pjrt.so")
    except Exception:
        hook = None
    mod = types.ModuleType("antenv.axon_hooks")
    state = {"hook": hook}
    mod.set_axon_ntff_profile_hook = lambda h: state.__setitem__("hook", h)
    mod.get_axon_ntff_profile_hook = lambda: state["hook"]
    sys.modules["antenv.axon_hooks"] = mod
    antenv.axon_hooks = mod


_ensure_ntff_hook()


def _build_nc():
    nc = bacc.Bacc()
    f32 = mybir.dt.float32
    bf16 = mybir.dt.bfloat16
    f8 = mybir.dt.float8e4
    AF = mybir.ActivationFunctionType
    ALU = mybir.AluOpType
    DR = mybir.MatmulPerfMode.DoubleRow

    xt_d = nc.dram_tensor("xt", [I_DIM, B_LOCAL], f32, kind="ExternalInput")
    w_d = nc.dram_tensor("w", [OH, NK_BF, P, ON], bf16, kind="ExternalInput")
    w8_d = nc.dram_tensor("w8", [OH, NPAIR, P, 2, ON], f8,
                          kind="ExternalInput")
    bias_d = nc.dram_tensor("bias", [P, O_DIM], f32, kind="ExternalInput")
    y_d = nc.dram_tensor("y", [B_LOCAL, O_DIM], f32, kind="ExternalOutput")

    with tile.TileContext(nc) as tc:
        with (
            tc.tile_pool(name="const", bufs=1) as cpool,
            tc.tile_pool(name="xin", bufs=2) as xpool,
            tc.tile_pool(name="fwork", bufs=2) as fpool,
            tc.tile_pool(name="basis", bufs=1) as bpool,
            tc.tile_pool(name="wstream", bufs=12) as wpool,
            tc.tile_pool(name="outbuf", bufs=4) as opool,
            tc.tile_pool(name="acc", bufs=1, space="PSUM") as ppool,
        ):
            # ---- PE warm-up ----
            # HAM un-throttles the PE clock only after sustained matmul
            # activity. Burn that window on dummy matmuls into psum bank 0
            # while the first xt/wt DMAs are in flight; the real k=0 matmul
            # re-starts the bank (start=True).
            warm = cpool.tile([P, ON], bf16, name="warm")
            # gpsimd clears its post-preamble barrier ~0.6us before DVE, so
            # memset there starts the warm-up (and the HAM activity timer)
            # earlier
            nc.gpsimd.memset(warm, 1.0)
            warm_ps = ppool.tile([P, ON], f32, tag="ps0", name="warm_ps")
            for wi in range(8):
                nc.tensor.matmul(warm_ps, warm[:, 0:P], warm,
                                 start=(wi == 0), stop=(wi == 7))

            # ---- basis production: T_1..T_8 per 128-row chunk of i ----
            basis = {}    # (ic, d) -> bf16 tile, ic < IC_BF
            pairs = {}    # (ic8, pr) -> fp8 pair tile, pr pairs (2pr+1,2pr+2)

            for ic in range(IC):
                is_f8 = ic >= IC_BF
                # ic == 0 runs every op on two half-tiles: the PE is already
                # warm when the kernel starts consuming, and half-granularity
                # lets the b<4 matmuls of each K-chunk start one half-op
                # earlier, which keeps the warm PE gapless during ramp-up.
                slices = ([slice(0, B_LOCAL // 2), slice(B_LOCAL // 2, B_LOCAL)]
                          if ic == 0 else [slice(0, B_LOCAL)])

                # xt on the HWDGE (sync) queue: issues in parallel with the
                # gpsimd wt stream and has lower first-byte latency.
                xt_t = xpool.tile([P, B_LOCAL], f32, tag="xt", name=f"xt_{ic}")
                for sl in slices:
                    nc.sync.dma_start(out=xt_t[:, sl],
                                      in_=xt_d[ic * P:(ic + 1) * P, sl])

                if is_f8:
                    pt = [bpool.tile([P, 2, B_LOCAL], f8, tag=f"p_{ic}_{pr}",
                                     name=f"p_{ic}_{pr}") for pr in range(4)]
                    for pr in range(4):
                        pairs[(ic - IC_BF, pr)] = pt[pr]
                    dsts = {d: pt[(d - 1) // 2][:, (d - 1) % 2, :]
                            for d in range(1, 9)}
                else:
                    def btile(d):
                        bt = bpool.tile([P, B_LOCAL], bf16, tag=f"b_{ic}_{d}",
                                        name=f"b_{ic}_{d}")
                        basis[(ic, d)] = bt
                        return bt
                    dsts = {d: btile(d) for d in range(1, 9)}

                # T1 = tanh(x) (no clip: the recurrence is stable for |t|<=1
                # and T_d(+-1) is finite; deviation from the reference's
                # clip at 0.999 is ~1e-6 on y)
                t = fpool.tile([P, B_LOCAL], f32, tag="T1", name=f"t_{ic}")
                s2 = fpool.tile([P, B_LOCAL], f32, tag="sq", name=f"s2_{ic}")
                T2 = fpool.tile([P, B_LOCAL], f32, tag="T2", name=f"T2_{ic}",
                                bufs=1)
                V3 = fpool.tile([P, B_LOCAL], f32, tag="u", name=f"V3_{ic}")
                T3 = fpool.tile([P, B_LOCAL], f32, tag="T3", name=f"T3_{ic}",
                                bufs=1)
                s4 = fpool.tile([P, B_LOCAL], f32, tag="sq", name=f"s4_{ic}")
                T4 = fpool.tile([P, B_LOCAL], f32, tag="T4", name=f"T4_{ic}",
                                bufs=1)
                s6 = fpool.tile([P, B_LOCAL], f32, tag="sq", name=f"s6_{ic}")
                s8 = fpool.tile([P, B_LOCAL], f32, tag="sq", name=f"s8_{ic}")
                if is_f8:
                    # leaves derived from the fp32 chain so each basis value
                    # carries exactly one e4m3 rounding (keeps the fp8 noise
                    # at the modeled level)
                    V5 = fpool.tile([P, B_LOCAL], f32, tag="u", name=f"V5_{ic}")
                    V7 = fpool.tile([P, B_LOCAL], f32, tag="u", name=f"V7_{ic}")
                else:
                    u5 = fpool.tile([P, B_LOCAL], bf16, tag="ub",
                                    name=f"u5_{ic}")
                    u7 = fpool.tile([P, B_LOCAL], bf16, tag="ub",
                                    name=f"u7_{ic}")
                b1, b2, b3, b4 = dsts[1], dsts[2], dsts[3], dsts[4]
                b5, b6, b7, b8 = dsts[5], dsts[6], dsts[7], dsts[8]

                for sl in slices:
                    nc.scalar.activation(t[:, sl], xt_t[:, sl], AF.Tanh)
                    # DVE cast: shortens the tanh -> first-matmul chain
                    nc.vector.tensor_copy(b1[:, sl], t[:, sl])

                    # T2 = 2 t^2 - 1
                    nc.scalar.square(s2[:, sl], t[:, sl])
                    nc.vector.tensor_scalar(T2[:, sl], s2[:, sl], 2.0, -1.0,
                                            ALU.mult, ALU.add)
                    nc.scalar.copy(b2[:, sl], T2[:, sl])

                    # T3 = 2 t T2 - t = t * (2 T2 - 1)
                    nc.vector.tensor_scalar(V3[:, sl], T2[:, sl], 2.0, -1.0,
                                            ALU.mult, ALU.add)
                    nc.vector.tensor_mul(T3[:, sl], t[:, sl], V3[:, sl])
                    nc.scalar.copy(b3[:, sl], T3[:, sl])

                    # T4 = 2 T2^2 - 1
                    nc.scalar.square(s4[:, sl], T2[:, sl])
                    nc.vector.tensor_scalar(T4[:, sl], s4[:, sl], 2.0, -1.0,
                                            ALU.mult, ALU.add)
                    nc.scalar.copy(b4[:, sl], T4[:, sl])

                    if is_f8:
                        # T5 = 2 T2 T3 - t, T7 = 2 T3 T4 - t from fp32
                        nc.vector.tensor_mul(V5[:, sl], T2[:, sl], T3[:, sl])
                        nc.vector.scalar_tensor_tensor(
                            b5[:, sl], V5[:, sl], 2.0, t[:, sl],
                            ALU.mult, ALU.subtract)
                        nc.scalar.square(s6[:, sl], T3[:, sl])
                        nc.vector.tensor_scalar(b6[:, sl], s6[:, sl],
                                                2.0, -1.0, ALU.mult, ALU.add)
                        nc.vector.tensor_mul(V7[:, sl], T3[:, sl], T4[:, sl])
                        nc.vector.scalar_tensor_tensor(
                            b7[:, sl], V7[:, sl], 2.0, t[:, sl],
                            ALU.mult, ALU.subtract)
                        nc.scalar.square(s8[:, sl], T4[:, sl])
                        nc.vector.tensor_scalar(b8[:, sl], s8[:, sl],
                                                2.0, -1.0, ALU.mult, ALU.add)
                    else:
                        # Degrees 5..8 are leaves (no downstream consumer), so
                        # they can be produced in cheaper precision/modes:
                        #   T5 = 2 T2 T3 - T1, T7 = 2 T3 T4 - T1 from bf16
                        #   operands (bf16 DVE ops run in 2x mode)
                        #   T6 = 2 T3^2 - 1, T8 = 2 T4^2 - 1 as one
                        #   tensor_scalar with direct bf16 output
                        nc.vector.tensor_mul(u5[:, sl], b2[:, sl], b3[:, sl])
                        nc.vector.scalar_tensor_tensor(
                            b5[:, sl], u5[:, sl], 2.0, b1[:, sl],
                            ALU.mult, ALU.subtract)

                        nc.scalar.square(s6[:, sl], T3[:, sl])
                        nc.vector.tensor_scalar(b6[:, sl], s6[:, sl],
                                                2.0, -1.0, ALU.mult, ALU.add)

                        nc.vector.tensor_mul(u7[:, sl], b3[:, sl], b4[:, sl])
                        nc.vector.scalar_tensor_tensor(
                            b7[:, sl], u7[:, sl], 2.0, b1[:, sl],
                            ALU.mult, ALU.subtract)

                        nc.scalar.square(s8[:, sl], T4[:, sl])
                        nc.vector.tensor_scalar(b8[:, sl], s8[:, sl],
                                                2.0, -1.0, ALU.mult, ALU.add)

            # bias is only consumed at the end of each o-half pass; load it
            # late so it doesn't delay the xt/wt streams.
            bias_t = cpool.tile([P, O_DIM], f32, name="bias_t")
            nc.sync.dma_start(out=bias_t, in_=bias_d[:, :])

            # ---- contraction: two o-half passes over all K ----
            psums = [ppool.tile([P, ON], f32, tag=f"ps{b}", name=f"ps{b}")
                     for b in range(B_LOCAL // P)]
            # pass 0: o-half 0, all 8 batch banks (overlaps basis production)
            # pass 1a/1b: o-half 1 split in two bank halves, so the first
            # half's bias-adds + stores overlap the second half's matmuls
            # and the final tail only drains 4 banks.
            passes = [(0, 0, 8), (1, 0, 4), (1, 4, 8)]
            for pi, (oh, blo, bhi) in enumerate(passes):
                # prefetch this pass's fp8 weight pairs on the sync queue
                # (idle here; gpsimd carries the bf16 stream). They're only
                # consumed at the end of the pass, so latency is hidden, and
                # having them all resident enables the bank-major fp8 loop.
                wtps = []
                for dp in range(NPAIR):
                    wtp = wpool.tile([P, 2, ON], f8, tag="wt8",
                                     name=f"wt8_{pi}_{dp}")
                    nc.sync.dma_start(out=wtp, in_=w8_d[oh, dp])
                    wtps.append(wtp)
                # bf16 chunks: ic 0..4, degrees 1..8
                for k in range(NK_BF):
                    ic, dm1 = divmod(k, DEG)
                    wt = wpool.tile([P, ON], bf16, tag="wt",
                                    name=f"wt_{pi}_{k}")
                    nc.gpsimd.dma_start(out=wt, in_=w_d[oh, k])
                    bt = basis[(ic, dm1 + 1)]
                    for b in range(blo, bhi):
                        nc.tensor.matmul(
                            psums[b],
                            bt[:, b * P:(b + 1) * P],
                            wt,
                            start=(k == 0),
                            stop=False,
                        )
                # fp8 DoubleRow pairs: ic 5..7, degree pairs (1,2)..(7,8).
                # Bank-major: each bank's accumulation stops NPAIR slots
                # before the next bank's, so its drain (and the next pass's
                # start=True matmuls) overlap the remaining banks' matmuls
                # instead of bunching up at the pass boundary.
                for b in range(blo, bhi):
                    for dp in range(NPAIR):
                        ic8, pr = divmod(dp, DEG // 2)
                        pt = pairs[(ic8, pr)]
                        nc.tensor.matmul(
                            psums[b],
                            pt[:, :, b * P:(b + 1) * P],
                            wtps[dp],
                            start=False,
                            stop=(dp == NPAIR - 1),
                            perf_mode=DR,
                        )
                for b in range(blo, bhi):
                    ot = opool.tile([P, ON], f32, tag="ot", name=f"ot_{pi}_{b}")
                    bias_sl = bias_t[:, oh * ON:(oh + 1) * ON]
                    if pi == 0 and b < 4:
                        # banks 0-3 gate pass 1a: drain them via ACT copy so
                        # the start=True matmuls aren't stuck behind the
                        # serial DVE bias-add chain; add bias in place later
                        # (overlaps the next pass). ACT applies the 2^-12
                        # weight descale for free (out = scale*in).
                        nc.scalar.activation(ot, psums[b], AF.Copy,
                                             scale=W_SINV)
                        nc.vector.tensor_add(ot, ot, bias_sl)
                        nc.sync.dma_start(
                            out=y_d[b * P:(b + 1) * P,
                                    oh * ON:(oh + 1) * ON],
                            in_=ot)
                    elif pi == len(passes) - 1:
                        # final pass: half-granularity add+store so the DMA
                        # of the first half overlaps the second half's add.
                        # DMA_DIRECT2D issue is ~0.6us flat, so halves, not
                        # quarters.
                        for hh in range(2):
                            hsl = slice(hh * (ON // 2), (hh + 1) * (ON // 2))
                            nc.vector.scalar_tensor_tensor(
                                ot[:, hsl], psums[b][:, hsl], W_SINV,
                                bias_sl[:, hsl], ALU.mult, ALU.add)
                            nc.sync.dma_start(
                                out=y_d[b * P:(b + 1) * P,
                                        oh * ON + hh * (ON // 2):
                                        oh * ON + (hh + 1) * (ON // 2)],
                                in_=ot[:, hsl])
                    else:
                        nc.vector.scalar_tensor_tensor(
                            ot, psums[b], W_SINV, bias_sl,
                            ALU.mult, ALU.add)
                        nc.sync.dma_start(
                            out=y_d[b * P:(b + 1) * P,
                                    oh * ON:(oh + 1) * ON],
                            in_=ot)
    nc.compile()  # bacc legalization: splits multi-sem waits (TRN2 allows 1)
    return nc


def _get_nc():
    global _nc
    if _nc is None:
        _nc = _build_nc()
    return _nc


def _prep_inputs(x, cheby_coeffs):
    x = np.asarray(x, dtype=np.float32)
    C = np.asarray(cheby_coeffs, dtype=np.float32)
    bf16 = ml_dtypes.bfloat16
    f8 = ml_dtypes.float8_e4m3

    Wd = C[:, :, 1:] * np.float32(W_SCALE)             # [I, O, 8], scaled

    # bf16 part: W[oh, k=(ic,d), p, on] = Wd[ic*128+p, oh*512+on, d]
    Wb = Wd[:IC_BF * P].reshape(IC_BF, P, OH, ON, DEG)
    Wb = np.transpose(Wb, (2, 0, 4, 1, 3))             # [oh, ic, d, p, on]
    Wb = np.ascontiguousarray(Wb.reshape(OH, NK_BF, P, ON)).astype(bf16)

    # fp8 part: W8[oh, dp=(ic8,pair), p, slot, on], degrees (2pr+1, 2pr+2)
    W8 = Wd[IC_BF * P:].reshape(IC_F8, P, OH, ON, DEG // 2, 2)
    W8 = np.transpose(W8, (2, 0, 4, 1, 5, 3))    # [oh, ic8, pair, p, slot, on]
    W8 = np.ascontiguousarray(
        W8.reshape(OH, NPAIR, P, 2, ON)).astype(f8)

    bias = C[:, :, 0].sum(axis=0, dtype=np.float64).astype(np.float32)
    bias_rep = np.ascontiguousarray(np.broadcast_to(bias, (P, O_DIM)))

    in_maps = []
    for c in range(N_CORES):
        xt = np.ascontiguousarray(x[c * B_LOCAL:(c + 1) * B_LOCAL, :].T)
        in_maps.append({"xt": xt, "w": Wb, "w8": W8, "bias": bias_rep})
    return in_maps


def kernel(x, cheby_coeffs):
    global last_results
    nc = _get_nc()
    in_maps = _prep_inputs(x, cheby_coeffs)
    last_results = run_bass_kernel_spmd(nc, in_maps,
                                        core_ids=list(range(N_CORES)))
    y = np.concatenate([r["y"] for r in last_results.results], axis=0)
    return y


# revision 9
# speedup vs baseline: 1.0159x; 1.0159x over previous
"""ChebyKAN layer kernel for 8x Trainium2 NeuronCores.

Computes y[b,o] = sum_{i,d} T_d(tanh(x[b,i])) * C[i,o,d], d = 0..8,
with T_d the Chebyshev polynomials, via:
  - batch sharded 8 ways (1024 rows/core)
  - device computes T_1..T_8 with Chebyshev product identities
    (fp32 DVE/ACT)
  - d=0 term (T_0 == 1) folded into a host-precomputed bias[o]
  - contraction as matmuls accumulating fp32 in PSUM, K = (i,d) of
    size 8192:
      * i-chunks 0..5 (K=6144): bf16 basis x bf16 weights
      * i-chunks 6..7 (K=2048): fp8-e4m3 basis x fp8-e4m3 weights in
        DoubleRow perf mode (2 K-chunks per PE pass, ~1.44x bf16
        throughput). Quantization noise of the fp8 quarter keeps the
        full-output max rel err ~1.5e-2 (measured on the fixed
        setup_inputs seed), under the 2e-2 gate.
  - all weights host-scaled by 2^12 (exact in bf16/fp8) so the fp8
    coeffs (~1e-4) sit in e4m3's normal range; the drain rescales by
    2^-12 before adding the bias.
  - x is transposed on host so the basis is produced directly in
    [K, batch] (lhsT) layout; no on-device transpose needed.

Self-contained: hardcodes all shapes for inputs
  x: [8192, 1024] f32, cheby_coeffs: [1024, 1024, 9] f32.
"""

import numpy as np
import ml_dtypes

import concourse.bass as bass
import concourse.mybir as mybir
import concourse.tile as tile
from concourse import bacc
from concourse.bass_utils import run_bass_kernel_spmd

P = 128
B_TOTAL = 8192
I_DIM = 1024
O_DIM = 1024
DEG = 8              # degrees 1..8 on device (d=0 folded into bias)
N_CORES = 8
B_LOCAL = B_TOTAL // N_CORES     # 1024
IC = I_DIM // P                  # 8 input chunks
IC_BF = 5                        # i-chunks contracted in bf16
IC_F8 = IC - IC_BF               # i-chunks contracted in dual-fp8
NK_BF = IC_BF * DEG              # 40 bf16 K-chunks of 128
NPAIR = IC_F8 * (DEG // 2)       # 12 fp8 DoubleRow pairs (K=256 each)
OH = 2                           # output halves (PSUM capacity: 8 banks)
ON = O_DIM // OH                 # 512
W_SCALE = 4096.0                 # pow2: exact in bf16; lifts fp8 coeffs
W_SINV = 1.0 / W_SCALE           # out of e4m3's subnormal range

_nc = None
last_results = None  # BassKernelResults of the most recent run (for profiling)


def _ensure_ntff_hook():
    """bass_utils' trace path imports antenv.axon_hooks unconditionally, but
    this agent image's antenv package lacks that module. Synthesize it (with
    the real libaxon NTFF hook when available) so a BASS_TRACE=1 run traces
    instead of crashing."""
    import sys
    import types

    try:
        import antenv.axon_hooks  # noqa: F401
        return
    except ImportError:
        pass
    try:
        import antenv
    except ImportError:
        return
    hook = None
    try:
        from trn_agent_boot.trn_boot import _ntff_profile_via_ctypes
        hook = _ntff_profile_via_ctypes("/opt/axon/libaxon_pjrt.so")
    except Exception:
        hook = None
    mod = types.ModuleType("antenv.axon_hooks")
    state = {"hook": hook}
    mod.set_axon_ntff_profile_hook = lambda h: state.__setitem__("hook", h)
    mod.get_axon_ntff_profile_hook = lambda: state["hook"]
    sys.modules["antenv.axon_hooks"] = mod
    antenv.axon_hooks = mod


_ensure_ntff_hook()


def _build_nc():
    nc = bacc.Bacc()
    f32 = mybir.dt.float32
    bf16 = mybir.dt.bfloat16
    f8 = mybir.dt.float8e4
    AF = mybir.ActivationFunctionType
    ALU = mybir.AluOpType
    DR = mybir.MatmulPerfMode.DoubleRow

    xt_d = nc.dram_tensor("xt", [I_DIM, B_LOCAL], f32, kind="ExternalInput")
    w_d = nc.dram_tensor("w", [OH, NK_BF, P, ON], bf16, kind="ExternalInput")
    w8_d = nc.dram_tensor("w8", [OH, NPAIR, P, 2, ON], f8,
                          kind="ExternalInput")
    bias_d = nc.dram_tensor("bias", [P, O_DIM], f32, kind="ExternalInput")
    y_d = nc.dram_tensor("y", [B_LOCAL, O_DIM], f32, kind="ExternalOutput")

    with tile.TileContext(nc) as tc:
        with (
            tc.tile_pool(name="const", bufs=1) as cpool,
            tc.tile_pool(name="xin", bufs=2) as xpool,
            tc.tile_pool(name="fwork", bufs=2) as fpool,
            tc.tile_pool(name="basis", bufs=1) as bpool,
            tc.tile_pool(name="wstream", bufs=12) as wpool,
            tc.tile_pool(name="outbuf", bufs=4) as opool,
            tc.tile_pool(name="acc", bufs=1, space="PSUM") as ppool,
        ):
            # ---- PE warm-up ----
            # HAM un-throttles the PE clock only after sustained matmul
            # activity. Burn that window on dummy matmuls into psum bank 0
            # while the first xt/wt DMAs are in flight; the real k=0 matmul
            # re-starts the bank (start=True).
            warm = cpool.tile([P, ON], bf16, name="warm")
            # memset on DVE: putting it on gpsimd instead delays the wt DMA
            # stream behind it on the gpsimd queue (~3.5us slower end-to-end)
            nc.vector.memset(warm, 1.0)
            warm_ps = ppool.tile([P, ON], f32, tag="ps0", name="warm_ps")
            for wi in range(8):
                nc.tensor.matmul(warm_ps, warm[:, 0:P], warm,
                                 start=(wi == 0), stop=(wi == 7))

            # ---- basis production: T_1..T_8 per 128-row chunk of i ----
            basis = {}    # (ic, d) -> bf16 tile, ic < IC_BF
            pairs = {}    # (ic8, pr) -> fp8 pair tile, pr pairs (2pr+1,2pr+2)

            for ic in range(IC):
                is_f8 = ic >= IC_BF
                # ic == 0 runs every op on two half-tiles: the PE is already
                # warm when the kernel starts consuming, and half-granularity
                # lets the b<4 matmuls of each K-chunk start one half-op
                # earlier, which keeps the warm PE gapless during ramp-up.
                slices = ([slice(0, B_LOCAL // 2), slice(B_LOCAL // 2, B_LOCAL)]
                          if ic == 0 else [slice(0, B_LOCAL)])

                # xt on the HWDGE (sync) queue: issues in parallel with the
                # gpsimd wt stream and has lower first-byte latency.
                xt_t = xpool.tile([P, B_LOCAL], f32, tag="xt", name=f"xt_{ic}")
                for sl in slices:
                    nc.sync.dma_start(out=xt_t[:, sl],
                                      in_=xt_d[ic * P:(ic + 1) * P, sl])

                if is_f8:
                    pt = [bpool.tile([P, 2, B_LOCAL], f8, tag=f"p_{ic}_{pr}",
                                     name=f"p_{ic}_{pr}") for pr in range(4)]
                    for pr in range(4):
                        pairs[(ic - IC_BF, pr)] = pt[pr]
                    dsts = {d: pt[(d - 1) // 2][:, (d - 1) % 2, :]
                            for d in range(1, 9)}
                else:
                    def btile(d):
                        bt = bpool.tile([P, B_LOCAL], bf16, tag=f"b_{ic}_{d}",
                                        name=f"b_{ic}_{d}")
                        basis[(ic, d)] = bt
                        return bt
                    dsts = {d: btile(d) for d in range(1, 9)}

                # T1 = tanh(x) (no clip: the recurrence is stable for |t|<=1
                # and T_d(+-1) is finite; deviation from the reference's
                # clip at 0.999 is ~1e-6 on y)
                t = fpool.tile([P, B_LOCAL], f32, tag="T1", name=f"t_{ic}")
                s2 = fpool.tile([P, B_LOCAL], f32, tag="sq", name=f"s2_{ic}")
                T2 = fpool.tile([P, B_LOCAL], f32, tag="T2", name=f"T2_{ic}",
                                bufs=1)
                V3 = fpool.tile([P, B_LOCAL], f32, tag="u", name=f"V3_{ic}")
                T3 = fpool.tile([P, B_LOCAL], f32, tag="T3", name=f"T3_{ic}",
                                bufs=1)
                s4 = fpool.tile([P, B_LOCAL], f32, tag="sq", name=f"s4_{ic}")
                T4 = fpool.tile([P, B_LOCAL], f32, tag="T4", name=f"T4_{ic}",
                                bufs=1)
                s6 = fpool.tile([P, B_LOCAL], f32, tag="sq", name=f"s6_{ic}")
                s8 = fpool.tile([P, B_LOCAL], f32, tag="sq", name=f"s8_{ic}")
                if is_f8:
                    # leaves derived from the fp32 chain so each basis value
                    # carries exactly one e4m3 rounding (keeps the fp8 noise
                    # at the modeled level)
                    V5 = fpool.tile([P, B_LOCAL], f32, tag="u", name=f"V5_{ic}")
                    V7 = fpool.tile([P, B_LOCAL], f32, tag="u", name=f"V7_{ic}")
                else:
                    u5 = fpool.tile([P, B_LOCAL], bf16, tag="ub",
                                    name=f"u5_{ic}")
                    u7 = fpool.tile([P, B_LOCAL], bf16, tag="ub",
                                    name=f"u7_{ic}")
                b1, b2, b3, b4 = dsts[1], dsts[2], dsts[3], dsts[4]
                b5, b6, b7, b8 = dsts[5], dsts[6], dsts[7], dsts[8]

                for sl in slices:
                    nc.scalar.activation(t[:, sl], xt_t[:, sl], AF.Tanh)
                    # DVE cast: shortens the tanh -> first-matmul chain
                    nc.vector.tensor_copy(b1[:, sl], t[:, sl])

                    # T2 = 2 t^2 - 1
                    nc.scalar.square(s2[:, sl], t[:, sl])
                    nc.vector.tensor_scalar(T2[:, sl], s2[:, sl], 2.0, -1.0,
                                            ALU.mult, ALU.add)
                    nc.scalar.copy(b2[:, sl], T2[:, sl])

                    # T3 = 2 t T2 - t = t * (2 T2 - 1)
                    nc.vector.tensor_scalar(V3[:, sl], T2[:, sl], 2.0, -1.0,
                                            ALU.mult, ALU.add)
                    nc.vector.tensor_mul(T3[:, sl], t[:, sl], V3[:, sl])
                    nc.scalar.copy(b3[:, sl], T3[:, sl])

                    # T4 = 2 T2^2 - 1
                    nc.scalar.square(s4[:, sl], T2[:, sl])
                    nc.vector.tensor_scalar(T4[:, sl], s4[:, sl], 2.0, -1.0,
                                            ALU.mult, ALU.add)
                    nc.scalar.copy(b4[:, sl], T4[:, sl])

                    if is_f8:
                        # T5 = 2 T2 T3 - t, T7 = 2 T3 T4 - t from fp32
                        nc.vector.tensor_mul(V5[:, sl], T2[:, sl], T3[:, sl])
                        nc.vector.scalar_tensor_tensor(
                            b5[:, sl], V5[:, sl], 2.0, t[:, sl],
                            ALU.mult, ALU.subtract)
                        nc.scalar.square(s6[:, sl], T3[:, sl])
                        nc.vector.tensor_scalar(b6[:, sl], s6[:, sl],
                                                2.0, -1.0, ALU.mult, ALU.add)
                        nc.vector.tensor_mul(V7[:, sl], T3[:, sl], T4[:, sl])
                        nc.vector.scalar_tensor_tensor(
                            b7[:, sl], V7[:, sl], 2.0, t[:, sl],
                            ALU.mult, ALU.subtract)
                        nc.scalar.square(s8[:, sl], T4[:, sl])
                        nc.vector.tensor_scalar(b8[:, sl], s8[:, sl],
                                                2.0, -1.0, ALU.mult, ALU.add)
                    else:
                        # Degrees 5..8 are leaves (no downstream consumer), so
                        # they can be produced in cheaper precision/modes:
                        #   T5 = 2 T2 T3 - T1, T7 = 2 T3 T4 - T1 from bf16
                        #   operands (bf16 DVE ops run in 2x mode)
                        #   T6 = 2 T3^2 - 1, T8 = 2 T4^2 - 1 as one
                        #   tensor_scalar with direct bf16 output
                        nc.vector.tensor_mul(u5[:, sl], b2[:, sl], b3[:, sl])
                        nc.vector.scalar_tensor_tensor(
                            b5[:, sl], u5[:, sl], 2.0, b1[:, sl],
                            ALU.mult, ALU.subtract)

                        nc.scalar.square(s6[:, sl], T3[:, sl])
                        nc.vector.tensor_scalar(b6[:, sl], s6[:, sl],
                                                2.0, -1.0, ALU.mult, ALU.add)

                        nc.vector.tensor_mul(u7[:, sl], b3[:, sl], b4[:, sl])
                        nc.vector.scalar_tensor_tensor(
                            b7[:, sl], u7[:, sl], 2.0, b1[:, sl],
                            ALU.mult, ALU.subtract)

                        nc.scalar.square(s8[:, sl], T4[:, sl])
                        nc.vector.tensor_scalar(b8[:, sl], s8[:, sl],
                                                2.0, -1.0, ALU.mult, ALU.add)

            # bias is only consumed at the end of each o-half pass; load it
            # late so it doesn't delay the xt/wt streams.
            bias_t = cpool.tile([P, O_DIM], f32, name="bias_t")
            nc.sync.dma_start(out=bias_t, in_=bias_d[:, :])

            # ---- contraction: two o-half passes over all K ----
            psums = [ppool.tile([P, ON], f32, tag=f"ps{b}", name=f"ps{b}")
                     for b in range(B_LOCAL // P)]
            # pass 0: o-half 0, all 8 batch banks (overlaps basis production)
            # pass 1a/1b: o-half 1 split in two bank halves, so the first
            # half's bias-adds + stores overlap the second half's matmuls
            # and the final tail only drains 4 banks.
            passes = [(0, 0, 8), (1, 0, 4), (1, 4, 8)]
            for pi, (oh, blo, bhi) in enumerate(passes):
                # prefetch this pass's fp8 weight pairs on the sync queue
                # (idle here; gpsimd carries the bf16 stream). They're only
                # consumed at the end of the pass, so latency is hidden, and
                # having them all resident enables the bank-major fp8 loop.
                wtps = []
                for dp in range(NPAIR):
                    wtp = wpool.tile([P, 2, ON], f8, tag="wt8",
                                     name=f"wt8_{pi}_{dp}")
                    nc.sync.dma_start(out=wtp, in_=w8_d[oh, dp])
                    wtps.append(wtp)
                # bf16 chunks: ic 0..4, degrees 1..8
                for k in range(NK_BF):
                    ic, dm1 = divmod(k, DEG)
                    wt = wpool.tile([P, ON], bf16, tag="wt",
                                    name=f"wt_{pi}_{k}")
                    nc.gpsimd.dma_start(out=wt, in_=w_d[oh, k])
                    bt = basis[(ic, dm1 + 1)]
                    for b in range(blo, bhi):
                        nc.tensor.matmul(
                            psums[b],
                            bt[:, b * P:(b + 1) * P],
                            wt,
                            start=(k == 0),
                            stop=False,
                        )
                # fp8 DoubleRow pairs: ic 5..7, degree pairs (1,2)..(7,8).
                # Bank-major: each bank's accumulation stops NPAIR slots
                # before the next bank's, so its drain (and the next pass's
                # start=True matmuls) overlap the remaining banks' matmuls
                # instead of bunching up at the pass boundary.
                for b in range(blo, bhi):
                    for dp in range(NPAIR):
                        ic8, pr = divmod(dp, DEG // 2)
                        pt = pairs[(ic8, pr)]
                        nc.tensor.matmul(
                            psums[b],
                            pt[:, :, b * P:(b + 1) * P],
                            wtps[dp],
                            start=False,
                            stop=(dp == NPAIR - 1),
                            perf_mode=DR,
                        )
                for b in range(blo, bhi):
                    ot = opool.tile([P, ON], f32, tag="ot", name=f"ot_{pi}_{b}")
                    bias_sl = bias_t[:, oh * ON:(oh + 1) * ON]
                    if pi == 0 and b < 4:
                        # banks 0-3 gate pass 1a: drain them via ACT copy so
                        # the start=True matmuls aren't stuck behind the
                        # serial DVE bias-add chain; add bias in place later
                        # (overlaps the next pass). ACT applies the 2^-12
                        # weight descale for free (out = scale*in).
                        nc.scalar.activation(ot, psums[b], AF.Copy,
                                             scale=W_SINV)
                        nc.vector.tensor_add(ot, ot, bias_sl)
                        nc.sync.dma_start(
                            out=y_d[b * P:(b + 1) * P,
                                    oh * ON:(oh + 1) * ON],
                            in_=ot)
                    elif pi == len(passes) - 1:
                        # final pass: half-granularity add+store so the DMA
                        # of the first half overlaps the second half's add.
                        # DMA_DIRECT2D issue is ~0.6us flat, so halves, not
                        # quarters.
                        for hh in range(2):
                            hsl = slice(hh * (ON // 2), (hh + 1) * (ON // 2))
                            nc.vector.scalar_tensor_tensor(
                                ot[:, hsl], psums[b][:, hsl], W_SINV,
                                bias_sl[:, hsl], ALU.mult, ALU.add)
                            nc.sync.dma_start(
                                out=y_d[b * P:(b + 1) * P,
                                        oh * ON + hh * (ON // 2):
                                        oh * ON + (hh + 1) * (ON // 2)],
                                in_=ot[:, hsl])
                    else:
                        nc.vector.scalar_tensor_tensor(
                            ot, psums[b], W_SINV, bias_sl,
                            ALU.mult, ALU.add)
                        nc.sync.dma_start(
                            out=y_d[b * P:(b + 1) * P,
                                    oh * ON:(oh + 1) * ON],
                            in_=ot)
    nc.compile()  # bacc legalization: splits multi-sem waits (TRN2 allows 1)
    return nc


def _get_nc():
    global _nc
    if _nc is None:
        _nc = _build_nc()
    return _nc


def _prep_inputs(x, cheby_coeffs):
    x = np.asarray(x, dtype=np.float32)
    C = np.asarray(cheby_coeffs, dtype=np.float32)
    bf16 = ml_dtypes.bfloat16
    f8 = ml_dtypes.float8_e4m3

    Wd = C[:, :, 1:] * np.float32(W_SCALE)             # [I, O, 8], scaled

    # bf16 part: W[oh, k=(ic,d), p, on] = Wd[ic*128+p, oh*512+on, d]
    Wb = Wd[:IC_BF * P].reshape(IC_BF, P, OH, ON, DEG)
    Wb = np.transpose(Wb, (2, 0, 4, 1, 3))             # [oh, ic, d, p, on]
    Wb = np.ascontiguousarray(Wb.reshape(OH, NK_BF, P, ON)).astype(bf16)

    # fp8 part: W8[oh, dp=(ic8,pair), p, slot, on], degrees (2pr+1, 2pr+2)
    W8 = Wd[IC_BF * P:].reshape(IC_F8, P, OH, ON, DEG // 2, 2)
    W8 = np.transpose(W8, (2, 0, 4, 1, 5, 3))    # [oh, ic8, pair, p, slot, on]
    W8 = np.ascontiguousarray(
        W8.reshape(OH, NPAIR, P, 2, ON)).astype(f8)

    bias = C[:, :, 0].sum(axis=0, dtype=np.float64).astype(np.float32)
    bias_rep = np.ascontiguousarray(np.broadcast_to(bias, (P, O_DIM)))

    in_maps = []
    for c in range(N_CORES):
        xt = np.ascontiguousarray(x[c * B_LOCAL:(c + 1) * B_LOCAL, :].T)
        in_maps.append({"xt": xt, "w": Wb, "w8": W8, "bias": bias_rep})
    return in_maps


def kernel(x, cheby_coeffs):
    global last_results
    nc = _get_nc()
    in_maps = _prep_inputs(x, cheby_coeffs)
    last_results = run_bass_kernel_spmd(nc, in_maps,
                                        core_ids=list(range(N_CORES)))
    y = np.concatenate([r["y"] for r in last_results.results], axis=0)
    return y


# revision 10
# speedup vs baseline: 1.0205x; 1.0045x over previous
"""ChebyKAN layer kernel for 8x Trainium2 NeuronCores.

Computes y[b,o] = sum_{i,d} T_d(tanh(x[b,i])) * C[i,o,d], d = 0..8,
with T_d the Chebyshev polynomials, via:
  - batch sharded 8 ways (1024 rows/core)
  - device computes T_1..T_8 with Chebyshev product identities
    (fp32 DVE/ACT)
  - d=0 term (T_0 == 1) folded into a host-precomputed bias[o]
  - contraction as matmuls accumulating fp32 in PSUM, K = (i,d) of
    size 8192:
      * i-chunks 0..4 (K=5120): bf16 basis x bf16 weights
      * i-chunks 5..7 (K=3072): fp8-e4m3 basis x fp8-e4m3 weights in
        DoubleRow perf mode (2 K-chunks per 512-cycle PE pass = 2x
        bf16 throughput, measured 216ns/pair at full clock).
        Quantization noise of the fp8 3/8ths keeps the full-output
        max rel err at 1.79e-2 (deterministic on the fixed
        setup_inputs seed), under the 2e-2 gate.
  - all weights host-scaled by 2^12 (exact in bf16/fp8) so the fp8
    coeffs (~1e-4) sit in e4m3's normal range; the drain rescales by
    2^-12 before adding the bias.
  - x is transposed on host so the basis is produced directly in
    [K, batch] (lhsT) layout; no on-device transpose needed.

Self-contained: hardcodes all shapes for inputs
  x: [8192, 1024] f32, cheby_coeffs: [1024, 1024, 9] f32.
"""

import numpy as np
import ml_dtypes

import concourse.bass as bass
import concourse.mybir as mybir
import concourse.tile as tile
from concourse import bacc
from concourse.bass_utils import run_bass_kernel_spmd

P = 128
B_TOTAL = 8192
I_DIM = 1024
O_DIM = 1024
DEG = 8              # degrees 1..8 on device (d=0 folded into bias)
N_CORES = 8
B_LOCAL = B_TOTAL // N_CORES     # 1024
IC = I_DIM // P                  # 8 input chunks
IC_BF = 5                        # i-chunks contracted in bf16
IC_F8 = IC - IC_BF               # i-chunks contracted in dual-fp8
NK_BF = IC_BF * DEG              # 40 bf16 K-chunks of 128
NPAIR = IC_F8 * (DEG // 2)       # 12 fp8 DoubleRow pairs (K=256 each)
OH = 2                           # output halves (PSUM capacity: 8 banks)
ON = O_DIM // OH                 # 512
W_SCALE = 4096.0                 # pow2: exact in bf16; lifts fp8 coeffs
W_SINV = 1.0 / W_SCALE           # out of e4m3's subnormal range

_nc = None
last_results = None  # BassKernelResults of the most recent run (for profiling)


def _ensure_ntff_hook():
    """bass_utils' trace path imports antenv.axon_hooks unconditionally, but
    this agent image's antenv package lacks that module. Synthesize it (with
    the real libaxon NTFF hook when available) so a BASS_TRACE=1 run traces
    instead of crashing."""
    import sys
    import types

    try:
        import antenv.axon_hooks  # noqa: F401
        return
    except ImportError:
        pass
    try:
        import antenv
    except ImportError:
        return
    hook = None
    try:
        from trn_agent_boot.trn_boot import _ntff_profile_via_ctypes
        hook = _ntff_profile_via_ctypes("/opt/axon/libaxon_pjrt.so")
    except Exception:
        hook = None
    mod = types.ModuleType("antenv.axon_hooks")
    state = {"hook": hook}
    mod.set_axon_ntff_profile_hook = lambda h: state.__setitem__("hook", h)
    mod.get_axon_ntff_profile_hook = lambda: state["hook"]
    sys.modules["antenv.axon_hooks"] = mod
    antenv.axon_hooks = mod


_ensure_ntff_hook()


def _build_nc():
    nc = bacc.Bacc()
    f32 = mybir.dt.float32
    bf16 = mybir.dt.bfloat16
    f8 = mybir.dt.float8e4
    AF = mybir.ActivationFunctionType
    ALU = mybir.AluOpType
    DR = mybir.MatmulPerfMode.DoubleRow

    xt_d = nc.dram_tensor("xt", [I_DIM, B_LOCAL], f32, kind="ExternalInput")
    w_d = nc.dram_tensor("w", [OH, NK_BF, P, ON], bf16, kind="ExternalInput")
    w8_d = nc.dram_tensor("w8", [OH, NPAIR, P, 2, ON], f8,
                          kind="ExternalInput")
    bias_d = nc.dram_tensor("bias", [P, O_DIM], f32, kind="ExternalInput")
    y_d = nc.dram_tensor("y", [B_LOCAL, O_DIM], f32, kind="ExternalOutput")

    with tile.TileContext(nc) as tc:
        with (
            tc.tile_pool(name="const", bufs=1) as cpool,
            tc.tile_pool(name="xin", bufs=2) as xpool,
            tc.tile_pool(name="fwork", bufs=2) as fpool,
            tc.tile_pool(name="basis", bufs=1) as bpool,
            tc.tile_pool(name="wstream", bufs=12) as wpool,
            tc.tile_pool(name="outbuf", bufs=4) as opool,
            tc.tile_pool(name="acc", bufs=1, space="PSUM") as ppool,
        ):
            # ---- PE warm-up ----
            # HAM un-throttles the PE clock only after sustained matmul
            # activity. Burn that window on dummy matmuls into psum bank 0
            # while the first xt/wt DMAs are in flight; the real k=0 matmul
            # re-starts the bank (start=True).
            warm = cpool.tile([P, ON], bf16, name="warm")
            # memset on DVE: putting it on gpsimd instead delays the wt DMA
            # stream behind it on the gpsimd queue (~3.5us slower end-to-end)
            nc.vector.memset(warm, 1.0)
            warm_ps = ppool.tile([P, ON], f32, tag="ps0", name="warm_ps")
            for wi in range(8):
                nc.tensor.matmul(warm_ps, warm[:, 0:P], warm,
                                 start=(wi == 0), stop=(wi == 7))

            # ---- basis production: T_1..T_8 per 128-row chunk of i ----
            basis = {}    # (ic, d) -> bf16 tile, ic < IC_BF
            pairs = {}    # (ic8, pr) -> fp8 pair tile, pr pairs (2pr+1,2pr+2)

            for ic in range(IC):
                is_f8 = ic >= IC_BF
                # ic == 0 runs every op on two half-tiles: the PE is already
                # warm when the kernel starts consuming, and half-granularity
                # lets the b<4 matmuls of each K-chunk start one half-op
                # earlier, which keeps the warm PE gapless during ramp-up.
                slices = ([slice(0, B_LOCAL // 2), slice(B_LOCAL // 2, B_LOCAL)]
                          if ic == 0 else [slice(0, B_LOCAL)])

                # xt on the HWDGE (sync) queue: issues in parallel with the
                # gpsimd wt stream and has lower first-byte latency.
                xt_t = xpool.tile([P, B_LOCAL], f32, tag="xt", name=f"xt_{ic}")
                for sl in slices:
                    nc.sync.dma_start(out=xt_t[:, sl],
                                      in_=xt_d[ic * P:(ic + 1) * P, sl])

                if is_f8:
                    pt = [bpool.tile([P, 2, B_LOCAL], f8, tag=f"p_{ic}_{pr}",
                                     name=f"p_{ic}_{pr}") for pr in range(4)]
                    for pr in range(4):
                        pairs[(ic - IC_BF, pr)] = pt[pr]
                    dsts = {d: pt[(d - 1) // 2][:, (d - 1) % 2, :]
                            for d in range(1, 9)}
                else:
                    def btile(d):
                        bt = bpool.tile([P, B_LOCAL], bf16, tag=f"b_{ic}_{d}",
                                        name=f"b_{ic}_{d}")
                        basis[(ic, d)] = bt
                        return bt
                    dsts = {d: btile(d) for d in range(1, 9)}

                # T1 = tanh(x) (no clip: the recurrence is stable for |t|<=1
                # and T_d(+-1) is finite; deviation from the reference's
                # clip at 0.999 is ~1e-6 on y)
                t = fpool.tile([P, B_LOCAL], f32, tag="T1", name=f"t_{ic}")
                s2 = fpool.tile([P, B_LOCAL], f32, tag="sq", name=f"s2_{ic}")
                T2 = fpool.tile([P, B_LOCAL], f32, tag="T2", name=f"T2_{ic}",
                                bufs=1)
                V3 = fpool.tile([P, B_LOCAL], f32, tag="u", name=f"V3_{ic}")
                T3 = fpool.tile([P, B_LOCAL], f32, tag="T3", name=f"T3_{ic}",
                                bufs=1)
                s4 = fpool.tile([P, B_LOCAL], f32, tag="sq", name=f"s4_{ic}")
                T4 = fpool.tile([P, B_LOCAL], f32, tag="T4", name=f"T4_{ic}",
                                bufs=1)
                s6 = fpool.tile([P, B_LOCAL], f32, tag="sq", name=f"s6_{ic}")
                s8 = fpool.tile([P, B_LOCAL], f32, tag="sq", name=f"s8_{ic}")
                if is_f8:
                    # leaves derived from the fp32 chain so each basis value
                    # carries exactly one e4m3 rounding (keeps the fp8 noise
                    # at the modeled level)
                    V5 = fpool.tile([P, B_LOCAL], f32, tag="u", name=f"V5_{ic}")
                    V7 = fpool.tile([P, B_LOCAL], f32, tag="u", name=f"V7_{ic}")
                else:
                    u5 = fpool.tile([P, B_LOCAL], bf16, tag="ub",
                                    name=f"u5_{ic}")
                    u7 = fpool.tile([P, B_LOCAL], bf16, tag="ub",
                                    name=f"u7_{ic}")
                b1, b2, b3, b4 = dsts[1], dsts[2], dsts[3], dsts[4]
                b5, b6, b7, b8 = dsts[5], dsts[6], dsts[7], dsts[8]

                for sl in slices:
                    nc.scalar.activation(t[:, sl], xt_t[:, sl], AF.Tanh)
                    # DVE cast: shortens the tanh -> first-matmul chain
                    nc.vector.tensor_copy(b1[:, sl], t[:, sl])

                    # T2 = 2 t^2 - 1
                    nc.scalar.square(s2[:, sl], t[:, sl])
                    nc.vector.tensor_scalar(T2[:, sl], s2[:, sl], 2.0, -1.0,
                                            ALU.mult, ALU.add)
                    nc.scalar.copy(b2[:, sl], T2[:, sl])

                    # T3 = 2 t T2 - t = t * (2 T2 - 1)
                    nc.vector.tensor_scalar(V3[:, sl], T2[:, sl], 2.0, -1.0,
                                            ALU.mult, ALU.add)
                    nc.vector.tensor_mul(T3[:, sl], t[:, sl], V3[:, sl])
                    nc.scalar.copy(b3[:, sl], T3[:, sl])

                    # T4 = 2 T2^2 - 1
                    nc.scalar.square(s4[:, sl], T2[:, sl])
                    nc.vector.tensor_scalar(T4[:, sl], s4[:, sl], 2.0, -1.0,
                                            ALU.mult, ALU.add)
                    nc.scalar.copy(b4[:, sl], T4[:, sl])

                    if is_f8:
                        # T5 = 2 T2 T3 - t, T7 = 2 T3 T4 - t from fp32
                        nc.vector.tensor_mul(V5[:, sl], T2[:, sl], T3[:, sl])
                        nc.vector.scalar_tensor_tensor(
                            b5[:, sl], V5[:, sl], 2.0, t[:, sl],
                            ALU.mult, ALU.subtract)
                        nc.scalar.square(s6[:, sl], T3[:, sl])
                        nc.vector.tensor_scalar(b6[:, sl], s6[:, sl],
                                                2.0, -1.0, ALU.mult, ALU.add)
                        nc.vector.tensor_mul(V7[:, sl], T3[:, sl], T4[:, sl])
                        nc.vector.scalar_tensor_tensor(
                            b7[:, sl], V7[:, sl], 2.0, t[:, sl],
                            ALU.mult, ALU.subtract)
                        nc.scalar.square(s8[:, sl], T4[:, sl])
                        nc.vector.tensor_scalar(b8[:, sl], s8[:, sl],
                                                2.0, -1.0, ALU.mult, ALU.add)
                    else:
                        # Degrees 5..8 are leaves (no downstream consumer), so
                        # they can be produced in cheaper precision/modes:
                        #   T5 = 2 T2 T3 - T1, T7 = 2 T3 T4 - T1 from bf16
                        #   operands (bf16 DVE ops run in 2x mode)
                        #   T6 = 2 T3^2 - 1, T8 = 2 T4^2 - 1 as one
                        #   tensor_scalar with direct bf16 output
                        nc.vector.tensor_mul(u5[:, sl], b2[:, sl], b3[:, sl])
                        nc.vector.scalar_tensor_tensor(
                            b5[:, sl], u5[:, sl], 2.0, b1[:, sl],
                            ALU.mult, ALU.subtract)

                        nc.scalar.square(s6[:, sl], T3[:, sl])
                        nc.vector.tensor_scalar(b6[:, sl], s6[:, sl],
                                                2.0, -1.0, ALU.mult, ALU.add)

                        nc.vector.tensor_mul(u7[:, sl], b3[:, sl], b4[:, sl])
                        nc.vector.scalar_tensor_tensor(
                            b7[:, sl], u7[:, sl], 2.0, b1[:, sl],
                            ALU.mult, ALU.subtract)

                        nc.scalar.square(s8[:, sl], T4[:, sl])
                        nc.vector.tensor_scalar(b8[:, sl], s8[:, sl],
                                                2.0, -1.0, ALU.mult, ALU.add)

            # bias is only consumed at the end of each o-half pass; load it
            # late so it doesn't delay the xt/wt streams.
            bias_t = cpool.tile([P, O_DIM], f32, name="bias_t")
            nc.sync.dma_start(out=bias_t, in_=bias_d[:, :])

            # ---- contraction: two o-half passes over all K ----
            psums = [ppool.tile([P, ON], f32, tag=f"ps{b}", name=f"ps{b}")
                     for b in range(B_LOCAL // P)]
            # pass 0: o-half 0, all 8 batch banks (overlaps basis production)
            # pass 1a/1b: o-half 1 split in two bank halves, so the first
            # half's bias-adds + stores overlap the second half's matmuls
            # and the final tail only drains 4 banks.
            passes = [(0, 0, 8), (1, 0, 4), (1, 4, 8)]
            for pi, (oh, blo, bhi) in enumerate(passes):
                # prefetch this pass's fp8 weight pairs on the sync queue
                # (idle here; gpsimd carries the bf16 stream). They're only
                # consumed at the end of the pass, so latency is hidden, and
                # having them all resident enables the bank-major fp8 loop.
                wtps = []
                for dp in range(NPAIR):
                    wtp = wpool.tile([P, 2, ON], f8, tag="wt8",
                                     name=f"wt8_{pi}_{dp}")
                    nc.sync.dma_start(out=wtp, in_=w8_d[oh, dp])
                    wtps.append(wtp)
                # bf16 chunks: ic 0..4, degrees 1..8
                for k in range(NK_BF):
                    ic, dm1 = divmod(k, DEG)
                    wt = wpool.tile([P, ON], bf16, tag="wt",
                                    name=f"wt_{pi}_{k}")
                    nc.gpsimd.dma_start(out=wt, in_=w_d[oh, k])
                    bt = basis[(ic, dm1 + 1)]
                    for b in range(blo, bhi):
                        nc.tensor.matmul(
                            psums[b],
                            bt[:, b * P:(b + 1) * P],
                            wt,
                            start=(k == 0),
                            stop=False,
                        )
                # fp8 DoubleRow pairs: ic 5..7, degree pairs (1,2)..(7,8).
                # Bank-major: each bank's accumulation stops NPAIR slots
                # before the next bank's, so its drain (and the next pass's
                # start=True matmuls) overlap the remaining banks' matmuls
                # instead of bunching up at the pass boundary.
                for b in range(blo, bhi):
                    for dp in range(NPAIR):
                        ic8, pr = divmod(dp, DEG // 2)
                        pt = pairs[(ic8, pr)]
                        nc.tensor.matmul(
                            psums[b],
                            pt[:, :, b * P:(b + 1) * P],
                            wtps[dp],
                            start=False,
                            stop=(dp == NPAIR - 1),
                            perf_mode=DR,
                        )
                for b in range(blo, bhi):
                    ot = opool.tile([P, ON], f32, tag="ot", name=f"ot_{pi}_{b}")
                    bias_sl = bias_t[:, oh * ON:(oh + 1) * ON]
                    if pi == 0 and b < 4:
                        # banks 0-3 gate pass 1a: drain them via ACT copy so
                        # the start=True matmuls aren't stuck behind the
                        # serial DVE bias-add chain; add bias in place later
                        # (overlaps the next pass). ACT applies the 2^-12
                        # weight descale for free (out = scale*in).
                        nc.scalar.activation(ot, psums[b], AF.Copy,
                                             scale=W_SINV)
                        nc.vector.tensor_add(ot, ot, bias_sl)
                        nc.sync.dma_start(
                            out=y_d[b * P:(b + 1) * P,
                                    oh * ON:(oh + 1) * ON],
                            in_=ot)
                    elif pi == len(passes) - 1:
                        # final pass: half-granularity add+store so the DMA
                        # of the first half overlaps the second half's add.
                        # DMA_DIRECT2D issue is ~0.6us flat, so halves, not
                        # quarters.
                        for hh in range(2):
                            hsl = slice(hh * (ON // 2), (hh + 1) * (ON // 2))
                            nc.vector.scalar_tensor_tensor(
                                ot[:, hsl], psums[b][:, hsl], W_SINV,
                                bias_sl[:, hsl], ALU.mult, ALU.add)
                            nc.sync.dma_start(
                                out=y_d[b * P:(b + 1) * P,
                                        oh * ON + hh * (ON // 2):
                                        oh * ON + (hh + 1) * (ON // 2)],
                                in_=ot[:, hsl])
                    else:
                        nc.vector.scalar_tensor_tensor(
                            ot, psums[b], W_SINV, bias_sl,
                            ALU.mult, ALU.add)
                        nc.sync.dma_start(
                            out=y_d[b * P:(b + 1) * P,
                                    oh * ON:(oh + 1) * ON],
                            in_=ot)
    nc.compile()  # bacc legalization: splits multi-sem waits (TRN2 allows 1)
    return nc


def _get_nc():
    global _nc
    if _nc is None:
        _nc = _build_nc()
    return _nc


def _prep_inputs(x, cheby_coeffs):
    x = np.asarray(x, dtype=np.float32)
    C = np.asarray(cheby_coeffs, dtype=np.float32)
    bf16 = ml_dtypes.bfloat16
    f8 = ml_dtypes.float8_e4m3

    Wd = C[:, :, 1:] * np.float32(W_SCALE)             # [I, O, 8], scaled

    # bf16 part: W[oh, k=(ic,d), p, on] = Wd[ic*128+p, oh*512+on, d]
    Wb = Wd[:IC_BF * P].reshape(IC_BF, P, OH, ON, DEG)
    Wb = np.transpose(Wb, (2, 0, 4, 1, 3))             # [oh, ic, d, p, on]
    Wb = np.ascontiguousarray(Wb.reshape(OH, NK_BF, P, ON)).astype(bf16)

    # fp8 part: W8[oh, dp=(ic8,pair), p, slot, on], degrees (2pr+1, 2pr+2)
    W8 = Wd[IC_BF * P:].reshape(IC_F8, P, OH, ON, DEG // 2, 2)
    W8 = np.transpose(W8, (2, 0, 4, 1, 5, 3))    # [oh, ic8, pair, p, slot, on]
    W8 = np.ascontiguousarray(
        W8.reshape(OH, NPAIR, P, 2, ON)).astype(f8)

    bias = C[:, :, 0].sum(axis=0, dtype=np.float64).astype(np.float32)
    bias_rep = np.ascontiguousarray(np.broadcast_to(bias, (P, O_DIM)))

    in_maps = []
    for c in range(N_CORES):
        xt = np.ascontiguousarray(x[c * B_LOCAL:(c + 1) * B_LOCAL, :].T)
        in_maps.append({"xt": xt, "w": Wb, "w8": W8, "bias": bias_rep})
    return in_maps


def kernel(x, cheby_coeffs):
    global last_results
    nc = _get_nc()
    in_maps = _prep_inputs(x, cheby_coeffs)
    last_results = run_bass_kernel_spmd(nc, in_maps,
                                        core_ids=list(range(N_CORES)))
    y = np.concatenate([r["y"] for r in last_results.results], axis=0)
    return y
